# revision 35
# baseline (speedup 1.0000x reference)
"""DistMatch (retrieval_knn) Trainium2 kernel — 8-core SPMD, bbox-pruned.

Problem (per batch group b of 4): for each of 8192 query points (int coords
in [0,128)^3), find the 5 candidates (of 8192) with smallest clipped L2
distance (ties -> lowest index, exactly like jax.lax.top_k), and accumulate
sigmoid-gated, distance-weighted candidate features.

Sharding: data-parallel over groups x query halves — core c handles group
c//2 and half c%2 of that group's (k-d sorted) queries.

Method:
  * Exact integer algebra: key = d2 + (2*orig_idx+1)/32768 is computed
    bit-exactly by one K=18 bf16 matmul per <=512 candidate columns
    (integer decomposition of coords/norms into bf16-exact rows).
  * Host k-d sorts queries (tiles of 128) and candidates (chunks of 64)
    and drops (qtile, chunk) pairs whose bounding boxes are > 16 apart.
    A rank-max slot schedule keeps the SPMD program identical across cores.
  * Software-pipelined device program: per bt-group, phase A (matmul +
    VectorE top-8 straight from PSUM), phase D (decode + fused index-wrap
    DMA + gather launch on rotating SWDGE queues with a ring big enough
    that descriptor generation never blocks on the DMA drain), phase C of
    bt-SKEW (one broadcast product + add-tree per half) interleaved.
  * RS candidate columns are stored grouped by (bt, half) so each half
    loads with a single DMA; ranked slots are dealt round-robin to
    bt-groups so each group carries ~1/4 of the candidate columns.
"""

import numpy as np
import ml_dtypes

B = 4
NA = 8192
NB = 8192
C = 112
CPAD = 128
TOPK = 5
NCORES = 8
QPC = NA // 2
CHW = 16  # candidate chunk width

BF16 = ml_dtypes.bfloat16
F32 = np.float32

_CACHE: dict = {}


# ---------------------------------------------------------------- host math
def _lhs_rows(ca):
    a = ca.astype(np.int64)
    ah, al = a >> 3, a & 7
    na2 = (a * a).sum(1)
    ma, ra = na2 >> 8, na2 & 255
    rows = np.zeros((18, a.shape[0]), np.float64)
    for d in range(3):
        rows[4 * d + 0] = ah[:, d]
        rows[4 * d + 1] = ah[:, d]
        rows[4 * d + 2] = al[:, d]
        rows[4 * d + 3] = al[:, d]
    rows[12] = ma
    rows[13] = ra
    rows[14:18] = 1.0
    return rows.astype(F32)


def _rhs_rows(cb):
    """[m,3] -> [18,m]; fraction rows encode the ORIGINAL candidate index."""
    b = cb.astype(np.int64)
    m = b.shape[0]
    bh, bl = b >> 3, b & 7
    nb2 = (b * b).sum(1)
    mb, rb = nb2 >> 8, nb2 & 255
    f = 2 * np.arange(m, dtype=np.int64) + 1
    fh, fl = f >> 6, f & 63
    r = np.zeros((18, m), np.float64)
    for d in range(3):
        r[4 * d + 0] = 128.0 * bh[:, d]
        r[4 * d + 1] = 16.0 * bl[:, d]
        r[4 * d + 2] = 16.0 * bh[:, d]
        r[4 * d + 3] = 2.0 * bl[:, d]
    r[12] = -256.0
    r[13] = -1.0
    r[14] = -256.0 * mb
    r[15] = -1.0 * rb
    r[16] = -(fh / 512.0)
    r[17] = -(fl / 32768.0)
    return r.astype(F32)


def _bf16(a):
    out = a.astype(BF16)
    assert np.array_equal(out.astype(F32), a)
    return out


def _scaled_feats(fb, w1, b1):
    fb = fb.astype(F32)
    z = fb @ w1.astype(F32) + b1.astype(F32)
    s = (1.0 / (1.0 + np.exp(-z, dtype=F32))).astype(F32)
    out = np.zeros((fb.shape[0], CPAD), F32)
    out[:, :C] = s * fb
    return out


def _kd_order(pts, leaf):
    out = []

    def rec(ids):
        if len(ids) <= leaf:
            out.append(ids)
            return
        p = pts[ids]
        dim = int(np.argmax(p.max(0) - p.min(0)))
        half = len(ids) // 2
        part = np.argpartition(p[:, dim], half)
        rec(ids[part[:half]])
        rec(ids[part[half:]])

    rec(np.arange(len(pts)))
    return np.concatenate(out)


def _plan_group(ca_g, cb_g):
    """k-d sort orders + per-half per-qtile surviving chunk lists."""
    pa = _kd_order(ca_g, 128)
    pb = _kd_order(cb_g, CHW)
    qa, qb = ca_g[pa], cb_g[pb]
    ct = qb.reshape(-1, CHW, 3)
    clo, chi = ct.min(1), ct.max(1)
    chunk_lists = []
    for h in range(2):
        qt = qa[h * QPC : (h + 1) * QPC].reshape(-1, 128, 3)
        qlo, qhi = qt.min(1), qt.max(1)
        lo = np.maximum(qlo[:, None, :], clo[None, :, :])
        hi = np.minimum(qhi[:, None, :], chi[None, :, :])
        gap = np.maximum(lo - hi, 0).astype(np.int64)
        keep = (gap**2).sum(-1) < 256
        chunk_lists.append([np.flatnonzero(keep[t]) for t in range(keep.shape[0])])
    return pa, pb, chunk_lists


def _make_caps(all_counts):
    nslots = len(all_counts[0])
    ranked = [sorted(c, reverse=True) for c in all_counts]
    return [max(1, max(r[t] for r in ranked)) for t in range(nslots)]


def _pack_core(chunks, caps, nch_total):
    order = np.argsort([-len(c) for c in chunks], kind="stable")
    slot_chunks = []
    for t, qt in enumerate(order):
        sel = list(chunks[qt])
        assert len(sel) <= caps[t]
        if len(sel) < caps[t]:
            selset = set(sel)
            pad = next(c for c in range(nch_total) if c not in selset)
            sel = sel + [pad] * (caps[t] - len(sel))
        slot_chunks.append(np.array(sel))
    return order, slot_chunks


# ---------------------------------------------------------------- device
def _build_program(nq, nb, caps, bqt=8):
    import concourse.tile as tile
    from concourse import bacc, mybir
    from concourse import hw_specs
    from concourse import library_config
    from concourse.tile_rust import add_dep_helper

    # Scheduler-calibration: the stock cost model prices SWDGE gather
    # descriptor generation at 0.34 ns/desc; measured unthrottled Q7
    # desc-gen for dma_gather is ~1.4 ns/idx. An honest number keeps the
    # Tile list-scheduler from placing gather-dependent vector work so
    # early that it head-of-line blocks the DVE stream.
    hw_specs.TRN2Spec.SWDGE_NS_PER_DESCRIPTOR = 1.4

    nqt = nq // 128
    nbt = nqt // bqt
    hq = bqt // 2
    assert nqt % bqt == 0 and len(caps) == nqt
    f32, bf16, i16 = mybir.dt.float32, mybir.dt.bfloat16, mybir.dt.int16
    TWO23 = float(2.0**23)
    AL = mybir.AluOpType
    AF = mybir.ActivationFunctionType
    AX = mybir.AxisListType
    rank = lambda bt, j: nbt * j + (nbt - 1 - bt)
    # RS columns are stored grouped by (bt, half): one DMA per half.
    wcol = [[caps[rank(bt, j)] * CHW for j in range(bqt)] for bt in range(nbt)]
    half_w = [
        [sum(wcol[bt][h * hq : (h + 1) * hq]) for h in range(2)] for bt in range(nbt)
    ]
    half_off = np.cumsum([0] + [half_w[bt][h] for bt in range(nbt) for h in range(2)])
    hwmax = max(max(hw) for hw in half_w)
    wmax = max(caps) * CHW
    nidx = hq * TOPK * 128  # gather rows per half (2560)

    nc = bacc.Bacc(
        "TRN2",
        target_bir_lowering=False,
        debug=False,
        num_swdge_queues=4,
        dynamic_dma_scratch_size=45056,
    )
    EYE = nc.dram_tensor("eye", [128, 128], f32, kind="ExternalInput")
    LT1 = nc.dram_tensor("lt1", [18, nq], bf16, kind="ExternalInput")
    RS = nc.dram_tensor("rs", [18, sum(c * CHW for c in caps)], bf16,
                        kind="ExternalInput")
    FBP = nc.dram_tensor("fbp", [nb, CPAD], f32, kind="ExternalInput")
    TMP = nc.dram_tensor("tmp", [nq, C], f32, kind="ExternalOutput")

    with tile.TileContext(nc) as tc:
        with (
            tc.tile_pool(name="const", bufs=1) as constp,
            tc.tile_pool(name="rstr", bufs=4) as rstrp,
            tc.tile_pool(name="cand", bufs=2) as candp,
            tc.tile_pool(name="small", bufs=4) as smallp,
            tc.tile_pool(name="wrap", bufs=4) as wrapp,
            tc.tile_pool(name="gath", bufs=3) as gathp,
            tc.tile_pool(name="prod", bufs=2) as prodp,
            tc.tile_pool(name="acc", bufs=2) as accp,
            tc.tile_pool(name="psum", bufs=2, space="PSUM") as psp,
        ):
            lib_inst = nc.gpsimd.load_library(library_config.mlp)

            lt1_sb = constp.tile([18, nq], bf16)
            nc.sync.dma_start(lt1_sb[:], LT1[:])
            eye_sb = constp.tile([128, 128], f32)
            nc.sync.dma_start(eye_sb[:], EYE[:])

            state = {}  # carries phase-C inputs from the previous bt

            def load_rs_half(bt, h):
                w = half_w[bt][h]
                off = half_off[2 * bt + h]
                rsb = rstrp.tile([18, hwmax], bf16, tag="rsb")
                nc.sync.dma_start(rsb[:, :w], RS[:, off : off + w])
                return rsb

            def a_tile(bt, j, rsb, top8):
                r = rank(bt, j)
                w_t = wcol[bt][j]
                loc = sum(wcol[bt][(j // hq) * hq : j])
                nps_t = (w_t + 1535) // 1536
                cand = None
                if nps_t > 1:
                    cand = candp.tile([128, 16], f32, tag="cand")
                for h in range(nps_t):
                    pw = min(1536, w_t - h * 1536)
                    ps = psp.tile([128, 1536], f32, tag="ps", bufs=2)
                    for cc in range(0, pw, 512):
                        mw = min(512, pw - cc)
                        nc.tensor.matmul(
                            ps[:, cc : cc + mw],
                            lt1_sb[:, r * 128 : (r + 1) * 128],
                            rsb[:, loc + h * 1536 + cc : loc + h * 1536 + cc + mw],
                            start=True,
                            stop=True,
                        )
                    if nps_t == 1:
                        nc.vector.max(top8[:, j, :], ps[:, :pw])
                    else:
                        nc.vector.max(cand[:, h * 8 : (h + 1) * 8], ps[:, :pw])
                if nps_t > 1:
                    # merge only the valid chunk results
                    nc.vector.max(top8[:, j, :], cand[:, : nps_t * 8])

            def c_half(bt, half, G, wgt, acc):
                # one broadcast product + add tree for the whole half
                jsl = slice(half * hq, (half + 1) * hq)
                prod = prodp.tile([128, hq, TOPK, C], f32, tag="prod")
                gv = G[half][:, :, :C].rearrange("p (j k) c -> p j k c", j=hq)
                wv = (
                    wgt[:, jsl, 0:TOPK]
                    .unsqueeze(-1)
                    .broadcast_to([128, hq, TOPK, C])
                )
                nc.vector.tensor_tensor(prod[:], gv, wv, AL.mult)
                t01 = prodp.tile([128, hq, C], f32, tag="t01", bufs=1)
                t23 = prodp.tile([128, hq, C], f32, tag="t23", bufs=1)
                nc.vector.tensor_tensor(
                    t01[:], prod[:, :, 0, :], prod[:, :, 1, :], AL.add
                )
                nc.vector.tensor_tensor(
                    t23[:], prod[:, :, 2, :], prod[:, :, 3, :], AL.add
                )
                nc.vector.tensor_tensor(t01[:], t01[:], t23[:], AL.add)
                nc.vector.tensor_tensor(
                    acc[:, jsl, :], t01[:], prod[:, :, 4, :], AL.add
                )

            def c_out(bt, acc):
                tmp_v = TMP[:].rearrange(
                    "(rj rb q) c -> rb q rj c", rj=bqt, rb=nbt, q=128
                )[nbt - 1 - bt]
                nc.scalar.dma_start(tmp_v, acc[:])

            def d_half(bt, half, top8, wgt):
                # decode keys for tiles j in [half*hq, (half+1)*hq):
                # top8 = -(d2 + frac), frac in (0, 0.5)
                qn = (bt % 2) * 2 + half  # SWDGE queue / wrap band
                jsl = slice(half * hq, (half + 1) * hq)
                t8 = top8[:, jsl, :].rearrange("p a b -> p (a b)")
                wide = [128, hq * 8]
                r1t = smallp.tile(wide, f32, tag=f"r1t{half}")
                nc.vector.tensor_scalar(r1t[:], t8, -1.0, TWO23, AL.mult, AL.add)
                rr = smallp.tile(wide, f32, tag=f"rr{half}")  # = d2
                nc.vector.tensor_scalar(rr[:], r1t[:], -TWO23, 0.0, AL.add, AL.add)
                ttm = smallp.tile(wide, f32, tag=f"ttm{half}")  # = -frac
                nc.vector.tensor_tensor(ttm[:], t8, rr[:], AL.add)
                jj = smallp.tile(wide, f32, tag=f"jj{half}")  # = orig index
                nc.vector.tensor_scalar(
                    jj[:], ttm[:], -16384.0, -0.5, AL.mult, AL.add
                )
                jc = smallp.tile([128, hq * TOPK], f32, tag=f"jc{half}")
                nc.vector.tensor_scalar(
                    jc[:],
                    jj[:].rearrange("p (a b) -> p a b", b=8)[:, :, 0:TOPK],
                    0.0, float(nb - 1),
                    AL.max, AL.min,
                )

                def emit_wgt():
                    sq = smallp.tile(wide, f32, tag=f"sq{half}")
                    nc.scalar.sqrt(sq[:], rr[:])
                    nc.scalar.activation(
                        wgt[:, jsl, :].rearrange("p a b -> p (a b)"), sq[:],
                        AF.Relu, bias=1.0, scale=-0.0625,
                    )

                # i16 index image: value of (query q, slot s=j*5+k) must land
                # at wrap[q%16 + 32*qn (+16 replica), s*8 + q//16]: queue
                # qn's Q7 cpu pair reads idx partitions [32qn, 32qn+32).
                # Built on-chip: PE transpose jc -> T[s, q], 8 PE block
                # transposes land W_a[p0, s] at PSUM partitions 0:16, one
                # Pool copy casts and permutes (a, s) -> (s*8+a) into a
                # 16-partition i16 staging image, and two small DMAs place
                # it at the queue's band (+ its replica).
                ns = hq * TOPK  # slots per half
                base = 32 * qn
                wrap = wrapp.tile([128, nidx // 16], i16, tag=f"wrap{half}")
                # one PSUM bank carries both transpose stages:
                # cols [0:128) = T[s, q], cols [128:288) = W_a blocks
                wp = psp.tile([128, 128 + 8 * ns], f32, tag="wps", bufs=2)
                tp_ps = wp[0:ns, 0:128]
                nc.tensor.transpose(tp_ps, jc[:], eye_sb[:])
                tsb = smallp.tile([ns, 128], f32, tag=f"tsb{half}")
                nc.scalar.mul(tsb[:], tp_ps, 1.0)
                wr_ps = wp[0:16, 128 : 128 + 8 * ns]
                for a in range(8):
                    nc.tensor.matmul(
                        wr_ps[:, ns * a : ns * (a + 1)],
                        tsb[:, 16 * a : 16 * (a + 1)],
                        eye_sb[0:ns, 0:ns],
                        start=True,
                        stop=True,
                        is_transpose=True,
                    )
                stg = smallp.tile([16, 8 * ns], i16, tag=f"stg{half}")
                nc.scalar.mul(
                    stg[:].rearrange("p (s a) -> p a s", a=8),
                    wr_ps.rearrange("p (a s) -> p a s", a=8),
                    1.0,
                )
                eng = nc.sync if half == 0 else nc.scalar
                eng.dma_start(wrap[base : base + 16, :], stg[:])
                rep = eng.dma_start(wrap[base + 16 : base + 32, :], stg[:])
                return (wrap, qn, rep), emit_wgt

            def launch_gather(bt, half, wrap, qn):
                Gh = gathp.tile([128, hq * TOPK, CPAD], f32, tag=f"G{half}")
                g_inst = nc.gpsimd.dma_gather(
                    Gh[:], FBP[:], wrap[:], nidx, nidx, CPAD,
                    single_packet=False, queue_num=qn,
                )
                add_dep_helper(
                    g_inst.ins, lib_inst.ins, True, "gather waits lib"
                )
                return Gh

            # software pipeline: iteration bt runs phase A+D of bt (decode +
            # gather launched per half-group) with phase C of bt-SKEW
            # interleaved after each half's gather is on its way.
            SKEW = 3
            for bt in range(nbt + SKEW):
                if bt < nbt:
                    top8 = smallp.tile([128, bqt, 8], f32, tag="top8")
                    wgt = smallp.tile([128, bqt, 8], f32, tag="wgt")
                    rsbs = [load_rs_half(bt, 0), load_rs_half(bt, 1)]
                    G = [None, None]
                prev = state.pop(bt - SKEW, None)
                if prev is not None:
                    acc = accp.tile([128, bqt, C], f32, tag="acc")
                if bt < nbt:
                    for j in range(bqt):
                        a_tile(bt, j, rsbs[j // hq], top8)
                    for half in range(2):
                        (wrap, qn, _), emit_wgt = d_half(bt, half, top8, wgt)
                        G[half] = launch_gather(bt, half, wrap, qn)
                        emit_wgt()
                if prev is not None:
                    for half in range(2):
                        c_half(bt - SKEW, half, *prev, acc)
                    c_out(bt - SKEW, acc)
                if bt < nbt:
                    state[bt] = (G, wgt)

    nc.compile()
    return nc


# ---------------------------------------------------------------- driver
def _prepare(coords_a, coords_b, feats_b, w1, b1):
    """Plan, build/compile (cached by caps), and produce per-core inputs.

    Returns (nc, in_maps, row_maps): row_maps[c] maps each output row of
    core c to its original query row within the core's group.
    """
    plans = [_plan_group(coords_a[g], coords_b[g]) for g in range(B)]
    all_counts = []
    for g in range(B):
        for h in range(2):
            all_counts.append([len(x) for x in plans[g][2][h]])
    caps = _make_caps(all_counts)

    key = tuple(caps)
    if _CACHE.get("key") != key:
        _CACHE["nc"] = _build_program(QPC, NB, caps)
        _CACHE["key"] = key
    nc = _CACHE["nc"]

    nqt = QPC // 128
    nbt = nqt // 8
    rank = lambda bt, j: nbt * j + (nbt - 1 - bt)
    # storage order of slots in RS: grouped by (bt, j)
    slot_order = [rank(bt, j) for bt in range(nbt) for j in range(8)]

    in_maps, row_maps = [], []
    for g in range(B):
        pa, pb, chunk_lists = plans[g]
        fbp = _scaled_feats(feats_b[g], w1, b1)
        rb_sorted = np.ascontiguousarray(_rhs_rows(coords_b[g])[:, pb])
        for h in range(2):
            qids = pa[h * QPC : (h + 1) * QPC]
            my_q = coords_a[g][qids]
            order, slot_chunks = _pack_core(chunk_lists[h], caps, NB // CHW)
            lt = _lhs_rows(my_q)
            lt_slots = np.concatenate(
                [lt[:, t * 128 : (t + 1) * 128] for t in order], axis=1
            )
            rs = np.concatenate(
                [rb_sorted[:, c0 * CHW : (c0 + 1) * CHW]
                 for r in slot_order for c0 in slot_chunks[r]],
                axis=1,
            )
            row_maps.append(
                np.concatenate([qids[t * 128 : (t + 1) * 128] for t in order])
            )
            in_maps.append(
                {
                    "eye": np.eye(128, dtype=F32),
                    "lt1": _bf16(lt_slots),
                    "rs": _bf16(np.ascontiguousarray(rs)),
                    "fbp": fbp,
                }
            )
    return nc, in_maps, row_maps


def kernel(coords_a, coords_b, feats_a, feats_b, w1, b1):
    from concourse.bass_utils import run_bass_kernel_spmd

    coords_a = np.asarray(coords_a)
    coords_b = np.asarray(coords_b)
    feats_a = np.asarray(feats_a, dtype=F32)
    feats_b = np.asarray(feats_b, dtype=F32)
    w1 = np.asarray(w1, dtype=F32)
    b1 = np.asarray(b1, dtype=F32)

    nc, in_maps, row_maps = _prepare(coords_a, coords_b, feats_b, w1, b1)
    res = run_bass_kernel_spmd(nc, in_maps, core_ids=list(range(NCORES)))

    out = np.empty((B, NA, 2 * C), F32)
    out[:, :, :C] = feats_a
    for c in range(NCORES):
        g = c // 2
        out[g][row_maps[c], C:] = res.results[c]["tmp"]
    return out
